# revision 24
# baseline (speedup 1.0000x reference)
"""Trainium2 Bass kernel: gated MoE residual block (two 3x3 convs, C=32).

  g  = gate * (gate > 0)                          # [B, C]
  h  = relu((conv3x3(x, w1) + b1) * g)
  h2 = relu((conv3x3(h, w2) + b2) * g)
  out = h2 + x

Sharding: data-parallel over batch. 16 images -> 8 cores x 2 images.

Device algorithm (per core, per image) — 2x2 space-to-depth conv in fp8
with DoubleRow matmuls:
  - x host-packed as x2[(2sr+sc)*32+ci, R', C'] = x[ci, 2R'-1+sr, 2C'-1+sc]
    (fp8 e4m3, odd-aligned 2x2 patches, zero halo baked in).
  - gating g and a fixed power-of-2 scale S=16 are folded into per-image
    fp8 weights (wv = fp8(S*g*w)), so every epilogue is a pure
    relu(psum + bias) -> one instruction on ScalarE or VectorE (GPSIMD
    cannot read PSUM on TRN2, so Pool only issues the output DMAs).
  - conv as 2 DoubleRow matmuls per PSUM group: the 2 k-tiles are the two
    row-alignments (ar), the 2 matmuls the column-alignments (ac).
    K = 2x(2x2 patch x 32ci), M = 2x2 phase x 32co.  rhs uses a flat
    overlapped AP [128, 2 (stride row), N (stride 1)] that runs across
    row boundaries: the junk lands only in a dead PSUM pad column.
    fp8 DoubleRow = 0.5 cycles/row -> 4x the bf16 matmul throughput.
  - PSUM slots hold TWO 3-row groups (2 banks, one group per bank); one
    fused epilogue instruction drains both, halving per-instr overheads.
  - h (= S * true h, fp8) is written in phase layout with halo; conv2
    reads it directly with even-aligned patches (odd output phases).
  - out (= S^2 * true h2, fp8; max ~204 < 240) staged and chunk-DMA'd;
    host de-interleaves, divides by S^2, adds the +x residual in fp32.
"""

import numpy as np
import ml_dtypes

import bass_rust
import concourse.bass as bass
import concourse.tile as tile
from concourse import bacc, mybir

B, C, H, W = 16, 32, 256, 256
IMGS_PER_CORE = 2
N_CORES = 8
F32 = mybir.dt.float32
BF16 = mybir.dt.bfloat16
FP8 = mybir.dt.float8e4
NP_FP8 = ml_dtypes.float8_e4m3

S = 16.0             # fp8 weight scale (power of 2; h stored as S*h)
G1 = H // 2          # 128 conv1 output groups per dim
GX = G1 + 1          # 129 x2 patch-grid size (odd-aligned, halo baked)
G2 = G1 + 1          # 129 conv2 output groups per dim (odd grid)
HW_ROW = GX + 1      # h_t row length 130 (left/right halo cols)
NQ = 10              # full quads of 3-row groups per conv (4 groups/slot)

DR = mybir.MatmulPerfMode.DoubleRow


def _pack_weights(w: np.ndarray) -> np.ndarray:
    """w: [C_out, C_in, 3, 3] (OIHW) -> [2, 2, 128, 128] lhsT[ar][ac].

    wv[ar, ac][(2sr+sc)*32+ci, (2qr+qc)*32+co] = w[co, ci, 2ar+sr-qr,
    2ac+sc-qc] when both taps lie in {0,1,2}.
    """
    wv = np.zeros((2, 2, 4 * C, 4 * C), dtype=np.float32)
    for ar in range(2):
        for ac in range(2):
            for sr in range(2):
                for sc in range(2):
                    for qr in range(2):
                        for qc in range(2):
                            dy = 2 * ar + sr - qr
                            dx = 2 * ac + sc - qc
                            if 0 <= dy <= 2 and 0 <= dx <= 2:
                                kb = (2 * sr + sc) * C
                                mb = (2 * qr + qc) * C
                                wv[ar, ac, kb:kb + C, mb:mb + C] = \
                                    w[:, :, dy, dx].T
    return wv


def _pack_x2(x: np.ndarray) -> np.ndarray:
    """x: [n, C, H, W] f32 -> [n, 128, 129, 129] fp8 odd 2x2 patches."""
    n = x.shape[0]
    xp = np.zeros((n, C, H + 4, W + 4), dtype=np.float32)
    xp[:, :, 1:1 + H, 1:1 + W] = x
    v = xp[:, :, 0:2 * GX, 0:2 * GX].reshape(n, C, GX, 2, GX, 2)
    x2 = v.transpose(0, 3, 5, 1, 2, 4).reshape(n, 4 * C, GX, GX)
    return np.ascontiguousarray(x2.astype(NP_FP8))


def _unpack_out(dev: np.ndarray) -> np.ndarray:
    """dev: [n, 128, 129, 129] (odd phases, S^2-scaled) -> [n,C,H,W] f32."""
    v = np.asarray(dev).astype(np.float32).reshape(-1, 2, 2, C, G2, G2)
    big = v.transpose(0, 3, 4, 1, 5, 2).reshape(-1, C, 2 * G2, 2 * G2)
    return np.ascontiguousarray(big[:, :, 1:1 + H, 1:1 + W]) * (1.0 / (S * S))


def _mk_ap(base, dims):
    """Custom AP with `base`'s tensor/offset/partition dim and free `dims`
    = [[stride, num], ...]."""
    a = base.copy()
    p = base.ap[0]
    a.ap = bass_rust.VecI64Pair([[p[0], p[1]]] + dims)
    return a


def _build_core_graph():
    nc = bacc.Bacc(None, target_bir_lowering=False, debug=False)

    x2_ext = nc.declare_dram_parameter("x2", [IMGS_PER_CORE, 4 * C, GX, GX], FP8, isOutput=False)
    wv1_ext = nc.declare_dram_parameter("wv1", [4 * C, IMGS_PER_CORE, 2, 2, 4 * C], FP8, isOutput=False)
    wv2_ext = nc.declare_dram_parameter("wv2", [4 * C, IMGS_PER_CORE, 2, 2, 4 * C], FP8, isOutput=False)
    bg1_ext = nc.declare_dram_parameter("bg1", [4 * C, IMGS_PER_CORE], F32, isOutput=False)
    bg2_ext = nc.declare_dram_parameter("bg2", [4 * C, IMGS_PER_CORE], F32, isOutput=False)
    out_ext = nc.declare_dram_parameter("out", [IMGS_PER_CORE, 4 * C, G2, G2], FP8, isOutput=True)

    RELU = mybir.ActivationFunctionType.Relu
    A_ADD = mybir.AluOpType.add
    A_MAX = mybir.AluOpType.max

    with tile.TileContext(nc) as tc:
        with (
            tc.tile_pool(name="const", bufs=1) as cpool,
            tc.tile_pool(name="xb", bufs=2) as xpool,
            tc.tile_pool(name="os", bufs=1) as ospool,
            tc.tile_pool(name="ps", bufs=4, space=bass.MemorySpace.PSUM) as pspool,
            tc.tile_pool(name="scr", bufs=3) as scrpool,
        ):
            wv1_t = cpool.tile([4 * C, IMGS_PER_CORE, 2, 2, 4 * C], FP8)
            wv2_t = cpool.tile([4 * C, IMGS_PER_CORE, 2, 2, 4 * C], FP8)
            bg1_t = cpool.tile([4 * C, IMGS_PER_CORE], F32)
            bg2_t = cpool.tile([4 * C, IMGS_PER_CORE], F32)
            # h rows 0 and 129 are the top/bottom halo; row 130 is scratch
            # absorbing the 1-element flat-AP overrun of the last tile
            h_t = cpool.tile([4 * C, HW_ROW + 1, HW_ROW], FP8)

            # PE warm-up: dummy matmuls start the clock ramp while DMAs
            # stream in; an early tiny activation pre-loads the Relu table
            warm = cpool.tile([4 * C, 512], BF16, tag="warm")
            warm8 = cpool.tile([4 * C, 16], BF16, tag="warm8")
            nc.vector.memset(warm[:], 0.0)
            nc.vector.memset(warm8[:], 0.0)
            wps = pspool.tile([4 * C, 2, 512], F32, tag="ps")
            for _ in range(6):
                nc.tensor.matmul(
                    wps[:, 0, 0:256], warm[:, 0:4 * C], warm[:, 0:256],
                    start=True, stop=True, skip_group_check=True)
            nc.scalar.activation(warm8[:], warm8[:], RELU)

            # weights first (first-needed), from the otherwise-idle Act SEQ
            nc.scalar.dma_start(out=wv1_t[:], in_=wv1_ext[:])
            nc.scalar.dma_start(out=wv2_t[:], in_=wv2_ext[:])
            # h halo borders + scratch row stay zero the whole kernel
            # (memsets on Pool, which has slack; DVE is an epilogue engine)
            nc.gpsimd.memset(h_t[:, 0, :], 0.0)
            nc.gpsimd.memset(h_t[:, GX, :], 0.0)
            nc.gpsimd.memset(h_t[:, HW_ROW, :], 0.0)
            nc.gpsimd.memset(h_t[:, :, 0], 0.0)
            nc.gpsimd.memset(h_t[:, :, HW_ROW - 1], 0.0)

            # x2 chunk bounds, first-need order (conv1 pair p needs rows
            # <= 6p+7); first chunks small so pair 0 starts ASAP
            xc = [0, 4, 12, 24, 40, 56, 72, 88, 104, 120, GX]
            # out store bounds (rows of out_t), flushed after pair p ends;
            # fine-grained at the end so the final store is tiny
            oc_ = [0, 24, 48, 72, 96, 120, G2]

            def conv_group(wv_t, img, src, r0, row_w, n, ps, j):
                """One 3-row output group: 2 DoubleRow matmuls into bank j."""
                out_flat = _mk_ap(ps[:, j:j + 1, 0:1], [[1, n]])
                for i, ac in enumerate((0, 1)):
                    rhs = _mk_ap(src[:, r0:r0 + 1, ac:ac + 1],
                                 [[row_w, 2], [1, n]])
                    nc.tensor.matmul(
                        out_flat, wv_t[:, img, ac, :, :], rhs,
                        start=(i == 0), stop=(i == 1),
                        perf_mode=DR, skip_group_check=True)

            def epilogue(eng_i, out_ap, in_ap, bg_ap):
                if eng_i == 0:
                    nc.scalar.activation(out_ap, in_ap, RELU, bias=bg_ap)
                else:
                    nc.vector.tensor_scalar(
                        out_ap, in_ap, bg_ap, 0.0, A_ADD, A_MAX)

            def pool_epilogue(ps, row_w, ncol, out_ap, bg_ap):
                """Pool cannot read PSUM: bounce the pair through SBUF via
                DMA (SP-issued), then relu(x+bg) on the Pool engine."""
                scr_t = scrpool.tile([4 * C, 2, row_w * 3], F32, tag="scr")
                nc.sync.dma_start(
                    out=scr_t[:],
                    in_=_mk_ap(ps[:, 0:1, 0:1], [[512, 2], [1, row_w * 3]]))
                nc.gpsimd.tensor_scalar(
                    out_ap,
                    _mk_ap(scr_t[:, 0:1, 0:1],
                           [[row_w * 3, 2], [row_w, 3], [1, ncol]]),
                    bg_ap, 0.0, A_ADD, A_MAX)

            # deficit-weighted Act/DVE interleave (Act is ~18% faster)
            def mk_assign(n, wa=1 / 788.0, wd=1 / 931.0):
                credit = [0.0, 0.0]
                out = []
                for _ in range(n):
                    credit[0] += wa / (wa + wd)
                    credit[1] += wd / (wa + wd)
                    i = 0 if credit[0] >= credit[1] else 1
                    credit[i] -= 1.0
                    out.append(i)
                return out

            NP = 21
            OFFLOAD = frozenset()     # PSUM->SBUF DMA unsupported: no Pool
            assign = mk_assign((NP + 1) * 2 * IMGS_PER_CORE + 2)
            ai = 0

            for img in range(IMGS_PER_CORE):
                # row 129 is scratch for the flat-AP overrun
                x2_t = xpool.tile([4 * C, GX + 1, GX], FP8)
                out_t = ospool.tile([4 * C, G2, G2], FP8)
                nc.gpsimd.memset(x2_t[:, GX, :], 0.0)

                for c0, c1 in zip(xc[:-1], xc[1:]):
                    nc.sync.dma_start(out=x2_t[:, c0:c1, :],
                                      in_=x2_ext[img, :, c0:c1, :])
                if img == 0:
                    nc.gpsimd.dma_start(out=bg1_t[:], in_=bg1_ext[:])
                    nc.gpsimd.dma_start(out=bg2_t[:], in_=bg2_ext[:])

                # ---- conv1: x2 -> h (even phases, +1 halo offset) ----
                # 21 pairs of 3-row groups, then one 2-row single
                for p in range(NP + 1):
                    ps = pspool.tile([4 * C, 2, 512], F32, tag="ps")
                    if p < NP:
                        for j in range(2):
                            conv_group(wv1_t, img, x2_t, 6 * p + 3 * j,
                                       GX, 3 * GX, ps, j)
                        if p in OFFLOAD:
                            pool_epilogue(ps, GX, G1,
                                          h_t[:, 1 + 6 * p:7 + 6 * p,
                                              1:1 + G1],
                                          bg1_t[:, img:img + 1])
                            continue
                        epilogue(assign[ai],
                                 h_t[:, 1 + 6 * p:7 + 6 * p, 1:1 + G1],
                                 _mk_ap(ps[:, 0:1, 0:1],
                                        [[512, 2], [GX, 3], [1, G1]]),
                                 bg1_t[:, img:img + 1])
                    else:
                        # rows 126..127: one 2-row group
                        conv_group(wv1_t, img, x2_t, 126, GX, 2 * GX, ps, 0)
                        epilogue(assign[ai],
                                 h_t[:, 127:129, 1:1 + G1],
                                 _mk_ap(ps[:, 0:1, 0:1], [[GX, 2], [1, G1]]),
                                 bg1_t[:, img:img + 1])
                    ai += 1

                # ---- conv2: h -> out_t (odd phases) + chunked stores ----
                # 21 pairs + one final 3-row single (rows 126..128)
                ostore = 0
                for p in range(NP + 1):
                    ps = pspool.tile([4 * C, 2, 512], F32, tag="ps")
                    if p < NP:
                        for j in range(2):
                            conv_group(wv2_t, img, h_t, 6 * p + 3 * j,
                                       HW_ROW, 3 * HW_ROW, ps, j)
                        hi = 6 * p + 6
                        if p in OFFLOAD:
                            pool_epilogue(ps, HW_ROW, G2,
                                          out_t[:, 6 * p:6 * p + 6, :],
                                          bg2_t[:, img:img + 1])
                        elif p == NP - 1:
                            # run concurrently with the Act single below
                            epilogue(1,
                                     out_t[:, 6 * p:6 * p + 6, :],
                                     _mk_ap(ps[:, 0:1, 0:1],
                                            [[512, 2], [HW_ROW, 3], [1, G2]]),
                                     bg2_t[:, img:img + 1])
                        else:
                            epilogue(assign[ai],
                                     out_t[:, 6 * p:6 * p + 6, :],
                                     _mk_ap(ps[:, 0:1, 0:1],
                                            [[512, 2], [HW_ROW, 3], [1, G2]]),
                                     bg2_t[:, img:img + 1])
                    else:
                        conv_group(wv2_t, img, h_t, 126, HW_ROW,
                                   3 * HW_ROW, ps, 0)
                        # last epilogue gates the final store: Act (faster)
                        epilogue(0,
                                 out_t[:, 126:129, :],
                                 _mk_ap(ps[:, 0:1, 0:1],
                                        [[HW_ROW, 3], [1, G2]]),
                                 bg2_t[:, img:img + 1])
                        hi = G2
                    ai += 1
                    if hi >= oc_[ostore + 1]:
                        a, b = oc_[ostore], oc_[ostore + 1]
                        nc.gpsimd.dma_start(out=out_ext[img, :, a:b, :],
                                            in_=out_t[:, a:b, :])
                        ostore += 1

    nc.compile()
    return nc


def _host_prep(x, gate_values, w1, b1, w2, b2):
    x = np.ascontiguousarray(np.asarray(x, dtype=np.float32))
    gate_values = np.asarray(gate_values, dtype=np.float32)
    w1 = np.asarray(w1, dtype=np.float32)
    b1 = np.asarray(b1, dtype=np.float32)
    w2 = np.asarray(w2, dtype=np.float32)
    b2 = np.asarray(b2, dtype=np.float32)

    g = gate_values * (gate_values > 0)                      # [B, C]

    in_maps = []
    for core in range(N_CORES):
        sl = slice(core * IMGS_PER_CORE, (core + 1) * IMGS_PER_CORE)
        gc = g[sl]                                           # [2, C]
        wv1 = np.zeros((4 * C, IMGS_PER_CORE, 2, 2, 4 * C), dtype=NP_FP8)
        wv2 = np.zeros_like(wv1)
        for img in range(IMGS_PER_CORE):
            p1 = _pack_weights(S * gc[img][:, None, None, None] * w1)
            p2 = _pack_weights(S * gc[img][:, None, None, None] * w2)
            for ac in range(2):
                for ar in range(2):
                    wv1[:, img, ac, ar, :] = p1[ar, ac].astype(NP_FP8)
                    wv2[:, img, ac, ar, :] = p2[ar, ac].astype(NP_FP8)
        in_maps.append({
            "x2": _pack_x2(x[sl]),
            "wv1": np.ascontiguousarray(wv1),
            "wv2": np.ascontiguousarray(wv2),
            "bg1": np.ascontiguousarray(np.tile(S * (gc * b1[None, :]).T, (4, 1))),
            "bg2": np.ascontiguousarray(np.tile(S * S * (gc * b2[None, :]).T, (4, 1))),
        })
    return in_maps


_NC_CACHE = None


def _get_graph():
    global _NC_CACHE
    if _NC_CACHE is None:
        _NC_CACHE = _build_core_graph()
    return _NC_CACHE


def kernel(x, gate_values, w1, b1, w2, b2, _trace=False, **_ignored):
    from concourse.bass_utils import run_bass_kernel_spmd

    nc = _get_graph()
    in_maps = _host_prep(x, gate_values, w1, b1, w2, b2)
    res = run_bass_kernel_spmd(
        nc, in_maps, core_ids=list(range(N_CORES)), trace=_trace)
    outs = [_unpack_out(res.results[i]["out"]) for i in range(N_CORES)]
    full = np.concatenate(outs, axis=0).astype(np.float32)
    full += np.asarray(x, dtype=np.float32)
    if _trace:
        return full, res
    return full
